# revision 6
# baseline (speedup 1.0000x reference)
"""Attention-Augmented Conv2D fused Bass kernel for 8 trn2 NeuronCores (v2).

Problem (hardcoded): x [4,64,32,32], NH=8, DK=DV=64, FILTERS=128 -> out [4,128,32,32].
Sharding: core c -> batch b=c//2, head-group g=c%2 (heads 4g..4g+4).
Each core produces:
  o_conv [64,512]  : conv1x1 output for its batch, positions [512g, 512g+512)
  o_attn [64,1024] : partial attn-out conv over its 4 heads (bias only on g==0)
Host gather: conv halves concatenated, attn partials summed per batch.

All projections fold their bias via a ones-row appended to x (x_aug [65,1024],
uploaded in bf16). Relative-position logits fold into the single logits matmul
with K-dim 72:
    KA_i = [D_w (32, k%32 indicator) ; D_h (32, k//32 indicator) ; K_i (8)]
    QA_i = [patwT (32) ; pathT (32) ; Q_i (8)]
patwT/pathT come from per-head rel-projections pf_w = (rel_w@Wq)^T@x_aug (on the
column-permuted q' = 32y+u index) and pf_h = (rel_h@Wq)^T@x_aug, staged through a
DRAM scratch and gathered back with a shifted (Toeplitz) access pattern.

Softmax skips max-subtraction (logits are O(few)); the denominator comes from a
ones-column in the V projection, so the PV matmul also produces the softmax
denominator (pv row 32i per head). The epilogue is per-head and overlaps the
next head's main loop: reciprocal (DVE) -> partition_broadcast (GPSIMD) ->
pv*rp (DVE, bf16); the wattn^T @ attn_n accumulation is deferred into the PE
stream at the next head's kt=6 so PE never stalls on the DVE chain.
"""
import sys
import numpy as np

sys.path.insert(0, '/opt/trn_rl_repo')

NH, DK, DV, FILTERS = 8, 64, 64, 128
B, C, H, W = 4, 64, 32, 32
HW = H * W
dkh = DK // NH
SCALE = dkh ** -0.5
N_CORES = 8


def _build_bass():
    import concourse.bass as bass
    import concourse.bacc as bacc
    import concourse.mybir as mybir
    import concourse.tile as tile

    f32 = mybir.dt.float32
    bf16 = mybir.dt.bfloat16
    AF = mybir.ActivationFunctionType

    nc = bacc.Bacc()

    xbf = nc.dram_tensor("xbf", [65, HW], bf16, kind="ExternalInput")
    xc = nc.dram_tensor("xc", [65, 512], bf16, kind="ExternalInput")
    wcat = nc.dram_tensor("wcat", [65, 668], bf16, kind="ExternalInput")
    wtail = nc.dram_tensor("wtail", [128, 64], bf16, kind="ExternalInput")
    dconst = nc.dram_tensor("dconst", [64, HW], bf16, kind="ExternalInput")
    o_conv = nc.dram_tensor("o_conv", [64, 512], f32, kind="ExternalOutput")
    o_attn = nc.dram_tensor("o_attn", [64, HW], f32, kind="ExternalOutput")
    scr_w = nc.dram_tensor("scr_w", [4, 64, HW], bf16)
    scr_h = nc.dram_tensor("scr_h", [4, 64, HW], bf16)

    with tile.TileContext(nc) as tc:
        with (
            tc.tile_pool(name="const", bufs=1) as constp,
            tc.tile_pool(name="kaqa", bufs=4) as kaqap,
            tc.tile_pool(name="pf", bufs=3) as pfp,
            tc.tile_pool(name="tmp", bufs=2) as tmpp,
            tc.tile_pool(name="pt", bufs=3) as ptp,
            tc.tile_pool(name="ep", bufs=2) as epp,
            tc.tile_pool(name="outp", bufs=1) as outp,
            tc.tile_pool(name="ps_lg", bufs=2, space="PSUM") as ps_lg,
            tc.tile_pool(name="ps_pf", bufs=1, space="PSUM") as ps_pf,
            tc.tile_pool(name="ps_pv", bufs=1, space="PSUM") as ps_pv,
        ):
            # ---- input DMAs, spread across the HW-DGE queues ----
            x_sb = constp.tile([65, HW], bf16, tag="x")
            nc.sync.dma_start(out=x_sb, in_=xbf[:, :])
            wcat_sb = constp.tile([65, 668], bf16, tag="wcat")
            nc.scalar.dma_start(out=wcat_sb, in_=wcat[:, :])
            xc_sb = constp.tile([65, 512], bf16, tag="xc")
            wqk_sb = wcat_sb[:, 0:64]
            wva_sb = wcat_sb[:, 64:100]
            wpatw_sb = wcat_sb[:, 100:352].rearrange("c (h m) -> c h m", h=4)
            wpath_sb = wcat_sb[:, 352:604].rearrange("c (h m) -> c h m", h=4)
            wconv_sb = wcat_sb[:, 604:668]
            wattn_sb = constp.tile([128, 64], bf16, tag="wtail")

            ka, qa = [], []
            for i in range(4):
                ka_i = kaqap.tile([72, HW], bf16, tag="ka", name=f"ka{i}")
                qa_i = kaqap.tile([72, HW], bf16, tag="qa", name=f"qa{i}")
                ka.append(ka_i)
                qa.append(qa_i)
            # gpsimd SWDGE queue: dconst first (needed by lg(0,0)), then the
            # late-use tensors (wtail at first oat ~21us, xc at the tail)
            nc.gpsimd.dma_start(out=ka[0][0:64, :], in_=dconst[:, :])
            nc.gpsimd.dma_start(out=wattn_sb, in_=wtail[:, :])
            nc.gpsimd.dma_start(out=xc_sb, in_=xc[:, :])

            # ---- qk projection ----
            qk_ps = ps_pv.tile([64, HW], f32, tag="v")
            for qc in range(2):
                nc.tensor.matmul(qk_ps[:, 512 * qc:512 * qc + 512],
                                 wqk_sb, x_sb[:, 512 * qc:512 * qc + 512])
            qk_bf = constp.tile([64, HW], bf16, tag="qk")
            nc.scalar.activation(qk_bf, qk_ps[:, :], AF.Copy)

            # ka/qa head rows + replicated dconst (gpsimd SWDGE, off critical path)
            for i in range(4):
                nc.gpsimd.dma_start(out=ka[i][64:72, :],
                                    in_=qk_bf[32 + 8 * i:40 + 8 * i, :])
                nc.gpsimd.dma_start(out=qa[i][64:72, :],
                                    in_=qk_bf[8 * i:8 * i + 8, :])
                if i > 0:
                    nc.gpsimd.dma_start(out=ka[i][0:64, :], in_=ka[0][0:64, :])

            # ---- per-head rel projections + scratch roundtrip gathers ----
            # pf tile: rows 0..63 = pf_w on q' (permuted) columns, 64..127 = pf_h
            xp = x_sb.rearrange("c (u y) -> c y u", y=32)
            for i in range(4):
                pf_ps = ps_pf.tile([127, HW], f32, tag="f")
                for qc in range(2):
                    nc.tensor.matmul(pf_ps[0:63, 512 * qc:512 * qc + 512],
                                     wpatw_sb[:, i, :],
                                     xp[:, 16 * qc:16 * qc + 16, :])
                for qc in range(2):
                    nc.tensor.matmul(pf_ps[64:127, 512 * qc:512 * qc + 512],
                                     wpath_sb[:, i, :],
                                     x_sb[:, 512 * qc:512 * qc + 512],
                                     tile_position=(0, 64))
                pfw_sb = pfp.tile([63, HW], bf16, tag="pf")
                pfh_sb = pfp.tile([63, HW], bf16, tag="pf")
                # pf_w copy on Act, pf_h copy on DVE (both free the psum slot)
                nc.scalar.activation(pfw_sb, pf_ps[0:63, :], AF.Copy)
                nc.vector.tensor_copy(out=pfh_sb, in_=pf_ps[64:127, :])
                # scratch writes + shifted gathers (SP HWDGE queue)
                nc.sync.dma_start(out=scr_w[i, 0:63, :], in_=pfw_sb)
                nc.sync.dma_start(out=scr_h[i, 0:63, :], in_=pfh_sb)
                tm = tmpp.tile([32, HW], bf16, tag="tm")
                # patwT[wk, (y,u)'] = pf_w[31+wk-y, .]: flat 31744+1024wk-992y+u
                nc.sync.dma_start(
                    out=tm.rearrange("p (y u) -> p y u", y=32),
                    in_=bass.AP(scr_w, i * 64 * HW + 31744,
                                [[1024, 32], [-992, 32], [1, 32]]))
                # pathT[hk, 32u+y] = pf_h[31+hk-u, q]: flat 31744+1024hk-992u+y
                nc.sync.dma_start(
                    out=qa[i][32:64, :].rearrange("p (u y) -> p u y", y=32),
                    in_=bass.AP(scr_h, i * 64 * HW + 31744,
                                [[1024, 32], [-992, 32], [1, 32]]))
                # unscramble q' -> q on DVE: qa[wk, 32u+y] = tm[wk, 32y+u]
                nc.vector.tensor_copy(
                    out=qa[i][0:32, :].rearrange("p (u y) -> p u y", y=32),
                    in_=tm.rearrange("p (y u) -> p u y", u=32))

            # ---- V^T projection ----
            vt_ps = ps_lg.tile([128, 8, 36], f32, tag="g")
            for kt in range(8):
                nc.tensor.matmul(vt_ps[:, kt, :],
                                 x_sb[:, 128 * kt:128 * kt + 128], wva_sb)
            vt_sb = constp.tile([128, 8, 36], bf16, tag="vt")
            nc.scalar.activation(vt_sb, vt_ps[:, :, :], AF.Copy)

            # ---- main attention loop ----
            pv_ps = ps_pv.tile([128, HW], f32, tag="v")
            attn_n = outp.tile([128, HW], bf16, tag="attn")
            oat_ps = ps_pf.tile([64, HW], f32, tag="f")
            oat_sb = outp.tile([64, HW], f32, tag="oat")
            conv_sb = outp.tile([64, 512], f32, tag="oconv")
            seq = [(i, kt) for i in range(4) for kt in range(8)]

            def emit_lg(i, kt):
                lg_ps = ps_lg.tile([128, HW], f32, tag="g")
                for qc in range(2):
                    nc.tensor.matmul(
                        lg_ps[:, 512 * qc:512 * qc + 512],
                        ka[i][:, 128 * kt:128 * kt + 128],
                        qa[i][:, 512 * qc:512 * qc + 512])
                return lg_ps

            def emit_norm(i):
                # DVE/GPSIMD only -- never blocks the PE stream
                rp = epp.tile([1, HW], f32, tag="rp")
                nc.vector.reciprocal(out=rp, in_=pv_ps[32 * i:32 * i + 1, :])
                rpb = epp.tile([9, HW], f32, tag="rpb")
                nc.gpsimd.partition_broadcast(rpb[0:9, :], rp[0:1, :])
                nc.vector.tensor_mul(attn_n[32 * i:32 * i + 9, :],
                                     pv_ps[32 * i:32 * i + 9, :], rpb[0:9, :])

            def emit_oat(i, stop):
                for qc in range(2):
                    nc.tensor.matmul(
                        oat_ps[0:64, 512 * qc:512 * qc + 512],
                        wattn_sb[32 * i:32 * i + 9, :],
                        attn_n[32 * i:32 * i + 9, 512 * qc:512 * qc + 512],
                        start=(i == 0), stop=stop,
                        tile_position=(32 * i, 0))

            lg_tiles = {seq[0]: emit_lg(*seq[0])}
            for j, (i, kt) in enumerate(seq):
                if j + 1 < len(seq):
                    lg_tiles[seq[j + 1]] = emit_lg(*seq[j + 1])
                lg_ps = lg_tiles.pop((i, kt))
                pt = ptp.tile([128, HW], bf16, tag="pt")
                nc.scalar.activation(pt, lg_ps[:, :], AF.Exp)
                for qc in range(2):
                    nc.tensor.matmul(
                        pv_ps[32 * i:32 * i + 9, 512 * qc:512 * qc + 512],
                        vt_sb[:, kt, 9 * i:9 * i + 9],
                        pt[:, 512 * qc:512 * qc + 512],
                        start=(kt == 0), stop=(kt == 7),
                        tile_position=(0, 32 * i))
                if kt == 6 and i >= 1:
                    emit_oat(i - 1, stop=False)  # prior head, mul_i surely done
                if kt == 7 and i < 3:
                    emit_norm(i)

            # ---- tail: conv (PE free now) + head-3 epilogue + outputs ----
            conv_ps = ps_lg.tile([64, 512], f32, tag="g")
            nc.tensor.matmul(conv_ps[:, :], wconv_sb, xc_sb)
            emit_norm(3)
            emit_oat(3, stop=True)
            nc.scalar.activation(conv_sb, conv_ps[:, :], AF.Copy)
            nc.sync.dma_start(out=o_conv[:, :], in_=conv_sb)
            nc.scalar.activation(oat_sb[:, 0:512], oat_ps[0:64, 0:512], AF.Copy)
            nc.sync.dma_start(out=o_attn[:, 0:512], in_=oat_sb[:, 0:512])
            nc.vector.tensor_copy(out=oat_sb[:, 512:HW], in_=oat_ps[0:64, 512:HW])
            nc.sync.dma_start(out=o_attn[:, 512:HW], in_=oat_sb[:, 512:HW])

    nc.compile()
    return nc


def _host_prep(inputs):
    import ml_dtypes
    bf = ml_dtypes.bfloat16
    x = np.ascontiguousarray(inputs['x'], np.float32)
    w_qkv = np.ascontiguousarray(inputs['w_qkv'].reshape(2 * DK + DV, C), np.float32)
    b_qkv = np.ascontiguousarray(inputs['b_qkv'], np.float32)
    w_conv = np.ascontiguousarray(inputs['w_conv'].reshape(FILTERS - DV, C), np.float32)
    b_conv = np.ascontiguousarray(inputs['b_conv'], np.float32)
    w_attn = np.ascontiguousarray(inputs['w_attn'].reshape(DV, DV), np.float32)
    b_attn = np.ascontiguousarray(inputs['b_attn'], np.float32)
    rel_h = np.ascontiguousarray(inputs['key_rel_h'], np.float32)  # [63, 8]
    rel_w = np.ascontiguousarray(inputs['key_rel_w'], np.float32)  # [63, 8]

    kk = np.arange(HW)
    DCmat = np.zeros((64, HW), np.float32)
    DCmat[:32] = (kk[None, :] % 32 == np.arange(32)[:, None])
    DCmat[32:] = (kk[None, :] // 32 == np.arange(32)[:, None])
    DCmat = DCmat.astype(bf)

    wconv_aug = np.concatenate([w_conv, b_conv[:, None]], 1).T  # [65, 64]

    in_maps = []
    for c in range(N_CORES):
        b, g = c // 2, c % 2
        heads = [4 * g + i for i in range(4)]
        x_aug = np.concatenate([x[b].reshape(C, HW),
                                np.ones((1, HW), np.float32)], 0)
        wq = w_qkv[32 * g:32 * g + 32] * SCALE
        bq = b_qkv[32 * g:32 * g + 32] * SCALE
        wk = w_qkv[64 + 32 * g:64 + 32 * g + 32]
        bk = b_qkv[64 + 32 * g:64 + 32 * g + 32]
        wqk_aug = np.concatenate(
            [np.concatenate([wq, wk], 0),
             np.concatenate([bq, bk], 0)[:, None]], 1).T  # [65, 64]
        wva_m = np.zeros((65, 36), np.float32)
        wpat_w = np.zeros((65, 4, 63), np.float32)
        wpat_h = np.zeros((65, 4, 63), np.float32)
        for i, h in enumerate(heads):
            wv = w_qkv[128 + 8 * h:128 + 8 * h + 8]
            bv = b_qkv[128 + 8 * h:128 + 8 * h + 8]
            wva_m[64, 9 * i] = 1.0
            wva_m[:64, 9 * i + 1:9 * i + 9] = wv.T
            wva_m[64, 9 * i + 1:9 * i + 9] = bv
            wq_h = w_qkv[8 * h:8 * h + 8] * SCALE
            bq_h = b_qkv[8 * h:8 * h + 8] * SCALE
            wpat_w[:64, i, :] = (rel_w @ wq_h).T
            wpat_w[64, i, :] = rel_w @ bq_h
            wpat_h[:64, i, :] = (rel_h @ wq_h).T
            wpat_h[64, i, :] = rel_h @ bq_h
        wattn_aug = np.zeros((128, 64), np.float32)
        for i, h in enumerate(heads):
            wattn_aug[32 * i + 1:32 * i + 9] = w_attn[:, 8 * h:8 * h + 8].T
        if g == 0:
            wattn_aug[0] += b_attn
        wcat = np.concatenate(
            [wqk_aug, wva_m, wpat_w.reshape(65, 252),
             wpat_h.reshape(65, 252), wconv_aug], 1)  # [65, 668]
        in_maps.append({
            'xbf': np.ascontiguousarray(x_aug.astype(bf)),
            'xc': np.ascontiguousarray(
                x_aug[:, 512 * g:512 * g + 512].astype(bf)),
            'wcat': np.ascontiguousarray(wcat.astype(bf)),
            'wtail': np.ascontiguousarray(wattn_aug.astype(bf)),
            'dconst': DCmat,
        })
    return in_maps


_CACHED = {}


def kernel(**inputs):
    from concourse.bass_utils import run_bass_kernel_spmd
    if 'nc' not in _CACHED:
        _CACHED['nc'] = _build_bass()
    nc = _CACHED['nc']
    in_maps = _host_prep(inputs)
    res = run_bass_kernel_spmd(nc, in_maps, core_ids=list(range(N_CORES)))
    out = np.zeros((B, FILTERS, HW), np.float32)
    for c in range(N_CORES):
        b, g = c // 2, c % 2
        out[b, :64, 512 * g:512 * g + 512] = res.results[c]['o_conv']
        out[b, 64:] += res.results[c]['o_attn']
    return out.reshape(B, FILTERS, H, W)


# revision 7
# speedup vs baseline: 1.0089x; 1.0089x over previous
"""Attention-Augmented Conv2D fused Bass kernel for 8 trn2 NeuronCores (v2).

Problem (hardcoded): x [4,64,32,32], NH=8, DK=DV=64, FILTERS=128 -> out [4,128,32,32].
Sharding: core c -> batch b=c//2, head-group g=c%2 (heads 4g..4g+4).
Each core produces:
  o_conv [64,512]  : conv1x1 output for its batch, positions [512g, 512g+512)
  o_attn [64,1024] : partial attn-out conv over its 4 heads (bias only on g==0)
Host gather: conv halves concatenated, attn partials summed per batch.

All projections fold their bias via a ones-row appended to x (x_aug [65,1024],
uploaded in bf16). Relative-position logits fold into the single logits matmul
with K-dim 72:
    KA_i = [D_w (32, k%32 indicator) ; D_h (32, k//32 indicator) ; K_i (8)]
    QA_i = [patwT (32) ; pathT (32) ; Q_i (8)]
patwT/pathT come from per-head rel-projections pf_w = (rel_w@Wq)^T@x_aug (on the
column-permuted q' = 32y+u index) and pf_h = (rel_h@Wq)^T@x_aug, staged through a
DRAM scratch and gathered back with a shifted (Toeplitz) access pattern.

Softmax skips max-subtraction (logits are O(few)); the denominator comes from a
ones-column in the V projection, so the PV matmul also produces the softmax
denominator (pv row 32i per head). The epilogue is per-head and overlaps the
next head's main loop: reciprocal (DVE) -> partition_broadcast (GPSIMD) ->
pv*rp (DVE, bf16); the wattn^T @ attn_n accumulation is deferred into the PE
stream at the next head's kt=6 so PE never stalls on the DVE chain.
"""
import sys
import numpy as np

sys.path.insert(0, '/opt/trn_rl_repo')

NH, DK, DV, FILTERS = 8, 64, 64, 128
B, C, H, W = 4, 64, 32, 32
HW = H * W
dkh = DK // NH
SCALE = dkh ** -0.5
N_CORES = 8


def _build_bass():
    import concourse.bass as bass
    import concourse.bacc as bacc
    import concourse.mybir as mybir
    import concourse.tile as tile

    f32 = mybir.dt.float32
    bf16 = mybir.dt.bfloat16
    AF = mybir.ActivationFunctionType

    nc = bacc.Bacc()

    xbf = nc.dram_tensor("xbf", [65, HW], bf16, kind="ExternalInput")
    xc = nc.dram_tensor("xc", [65, 512], bf16, kind="ExternalInput")
    wcat = nc.dram_tensor("wcat", [65, 668], bf16, kind="ExternalInput")
    wtail = nc.dram_tensor("wtail", [128, 64], bf16, kind="ExternalInput")
    dconst = nc.dram_tensor("dconst", [64, HW], bf16, kind="ExternalInput")
    o_conv = nc.dram_tensor("o_conv", [64, 512], f32, kind="ExternalOutput")
    o_attn = nc.dram_tensor("o_attn", [64, HW], f32, kind="ExternalOutput")
    scr_w = nc.dram_tensor("scr_w", [4, 64, HW], bf16)
    scr_h = nc.dram_tensor("scr_h", [4, 64, HW], bf16)

    with tile.TileContext(nc) as tc:
        with (
            tc.tile_pool(name="const", bufs=1) as constp,
            tc.tile_pool(name="kaqa", bufs=4) as kaqap,
            tc.tile_pool(name="pf", bufs=3) as pfp,
            tc.tile_pool(name="tmp", bufs=2) as tmpp,
            tc.tile_pool(name="pt", bufs=3) as ptp,
            tc.tile_pool(name="ep", bufs=2) as epp,
            tc.tile_pool(name="outp", bufs=1) as outp,
            tc.tile_pool(name="ps_lg", bufs=2, space="PSUM") as ps_lg,
            tc.tile_pool(name="ps_pf", bufs=1, space="PSUM") as ps_pf,
            tc.tile_pool(name="ps_pv", bufs=1, space="PSUM") as ps_pv,
        ):
            # ---- input DMAs, spread across the HW-DGE queues ----
            x_sb = constp.tile([65, HW], bf16, tag="x")
            for qc in range(2):
                nc.sync.dma_start(out=x_sb[:, 512 * qc:512 * qc + 512],
                                  in_=xbf[:, 512 * qc:512 * qc + 512])
            wcat_sb = constp.tile([65, 668], bf16, tag="wcat")
            nc.scalar.dma_start(out=wcat_sb, in_=wcat[:, :])
            xc_sb = constp.tile([65, 512], bf16, tag="xc")
            wqk_sb = wcat_sb[:, 0:64]
            wva_sb = wcat_sb[:, 64:100]
            wpatw_sb = wcat_sb[:, 100:352].rearrange("c (h m) -> c h m", h=4)
            wpath_sb = wcat_sb[:, 352:604].rearrange("c (h m) -> c h m", h=4)
            wconv_sb = wcat_sb[:, 604:668]
            wattn_sb = constp.tile([128, 64], bf16, tag="wtail")

            ka, qa = [], []
            for i in range(4):
                ka_i = kaqap.tile([72, HW], bf16, tag="ka", name=f"ka{i}")
                qa_i = kaqap.tile([72, HW], bf16, tag="qa", name=f"qa{i}")
                ka.append(ka_i)
                qa.append(qa_i)
            # gpsimd SWDGE queue: dconst first (needed by lg(0,0))
            nc.gpsimd.dma_start(out=ka[0][0:64, :], in_=dconst[:, :])

            # ---- PE warmup: start the p-state ramp clock at t~0 so the real
            # projections run at full clock (~3us of continuous PE activity)
            warm_s = constp.tile([1, 16], f32, tag="warms")
            warm_m = constp.tile([1, 512], f32, tag="warmm")
            nc.vector.memset(warm_s, 0.0)
            nc.vector.memset(warm_m, 0.0)
            warm_ps = ps_lg.tile([16, 512], f32, tag="g")
            for _ in range(2):
                nc.tensor.matmul(warm_ps[:, :], warm_s[:, 0:16], warm_m[:, :])
            # preload the activation table while Act waits for inputs
            warm_a = constp.tile([1, 16], f32, tag="warma")
            nc.scalar.activation(warm_a, warm_s[:, :], AF.Exp)

            # ---- qk projection ----
            qk_ps = ps_pv.tile([64, HW], f32, tag="v")
            for qc in range(2):
                nc.tensor.matmul(qk_ps[:, 512 * qc:512 * qc + 512],
                                 wqk_sb, x_sb[:, 512 * qc:512 * qc + 512])

            xp = x_sb.rearrange("c (u y) -> c y u", y=32)
            pf_pss, pfw_sbs, pfh_sbs, tms = [], [], [], []

            def emit_pf_mms(i):
                pf_ps = ps_pf.tile([127, HW], f32, tag="f", name=f"pf{i}")
                pf_pss.append(pf_ps)
                for qc in range(2):
                    nc.tensor.matmul(pf_ps[0:63, 512 * qc:512 * qc + 512],
                                     wpatw_sb[:, i, :],
                                     xp[:, 16 * qc:16 * qc + 16, :])
                for qc in range(2):
                    nc.tensor.matmul(pf_ps[64:127, 512 * qc:512 * qc + 512],
                                     wpath_sb[:, i, :],
                                     x_sb[:, 512 * qc:512 * qc + 512],
                                     tile_position=(0, 64))

            def emit_pf_copies(i, w_eng, h_eng):
                pfw_sb = pfp.tile([63, HW], bf16, tag="pf", name=f"pfw{i}")
                pfh_sb = pfp.tile([63, HW], bf16, tag="pf", name=f"pfh{i}")
                pfw_sbs.append(pfw_sb)
                pfh_sbs.append(pfh_sb)
                if w_eng == 'act':
                    nc.scalar.activation(pfw_sb, pf_pss[i][0:63, :], AF.Copy)
                else:
                    nc.vector.tensor_copy(out=pfw_sb, in_=pf_pss[i][0:63, :])
                nc.vector.tensor_copy(out=pfh_sb, in_=pf_pss[i][64:127, :])

            def emit_scratch(i):
                # scratch writes + shifted gathers (SP HWDGE queue)
                nc.sync.dma_start(out=scr_w[i, 0:63, :], in_=pfw_sbs[i])
                nc.sync.dma_start(out=scr_h[i, 0:63, :], in_=pfh_sbs[i])
                tm = tmpp.tile([32, HW], bf16, tag="tm", name=f"tm{i}")
                tms.append(tm)
                # patwT[wk, (y,u)'] = pf_w[31+wk-y, .]: flat 31744+1024wk-992y+u
                nc.sync.dma_start(
                    out=tm.rearrange("p (y u) -> p y u", y=32),
                    in_=bass.AP(scr_w, i * 64 * HW + 31744,
                                [[1024, 32], [-992, 32], [1, 32]]))
                # pathT[hk, 32u+y] = pf_h[31+hk-u, q]: flat 31744+1024hk-992u+y
                nc.sync.dma_start(
                    out=qa[i][32:64, :].rearrange("p (u y) -> p u y", y=32),
                    in_=bass.AP(scr_h, i * 64 * HW + 31744,
                                [[1024, 32], [-992, 32], [1, 32]]))

            def emit_unscr(i):
                # unscramble q' -> q on DVE: qa[wk, 32u+y] = tm[wk, 32y+u]
                nc.vector.tensor_copy(
                    out=qa[i][0:32, :].rearrange("p (u y) -> p u y", y=32),
                    in_=tms[i].rearrange("p (y u) -> p u y", u=32))

            def emit_qk_rows(i):
                nc.gpsimd.dma_start(out=ka[i][64:72, :],
                                    in_=qk_bf[32 + 8 * i:40 + 8 * i, :])
                nc.gpsimd.dma_start(out=qa[i][64:72, :],
                                    in_=qk_bf[8 * i:8 * i + 8, :])

            # head 0 critical chain first
            emit_pf_mms(0)
            emit_pf_copies(0, 'act', 'dve')
            emit_scratch(0)
            emit_unscr(0)
            qk_bf = constp.tile([64, HW], bf16, tag="qk")
            nc.scalar.activation(qk_bf, qk_ps[:, :], AF.Copy)
            emit_qk_rows(0)
            # head 1
            emit_pf_mms(1)
            emit_pf_copies(1, 'act', 'dve')
            emit_scratch(1)
            emit_unscr(1)
            nc.gpsimd.dma_start(out=ka[1][0:64, :], in_=ka[0][0:64, :])
            emit_qk_rows(1)
            # V^T projection (needed at pv(0,0), shortly after lg(0,0))
            vt_ps = ps_lg.tile([128, 8, 36], f32, tag="g")
            for kt in range(8):
                nc.tensor.matmul(vt_ps[:, kt, :],
                                 x_sb[:, 128 * kt:128 * kt + 128], wva_sb)
            vt_sb = constp.tile([128, 8, 36], bf16, tag="vt")
            nc.scalar.activation(vt_sb, vt_ps[:, :, :], AF.Copy)
            # heads 2, 3: everything off the Act engine (copies on DVE)
            nc.gpsimd.dma_start(out=wattn_sb, in_=wtail[:, :])
            for i in (2, 3):
                emit_pf_mms(i)
                emit_pf_copies(i, 'dve', 'dve')
                emit_scratch(i)
                emit_unscr(i)
                nc.gpsimd.dma_start(out=ka[i][0:64, :], in_=ka[0][0:64, :])
                emit_qk_rows(i)
            nc.gpsimd.dma_start(out=xc_sb, in_=xc[:, :])

            # ---- main attention loop ----
            pv_ps = ps_pv.tile([128, HW], f32, tag="v")
            attn_n = outp.tile([128, HW], bf16, tag="attn")
            oat_ps = ps_pf.tile([64, HW], f32, tag="f")
            oat_sb = outp.tile([64, HW], f32, tag="oat")
            conv_sb = outp.tile([64, 512], f32, tag="oconv")
            seq = [(i, kt) for i in range(4) for kt in range(8)]

            def emit_lg(i, kt):
                lg_ps = ps_lg.tile([128, HW], f32, tag="g")
                for qc in range(2):
                    nc.tensor.matmul(
                        lg_ps[:, 512 * qc:512 * qc + 512],
                        ka[i][:, 128 * kt:128 * kt + 128],
                        qa[i][:, 512 * qc:512 * qc + 512])
                return lg_ps

            def emit_norm(i):
                # DVE/GPSIMD only -- never blocks the PE stream
                rp = epp.tile([1, HW], f32, tag="rp")
                nc.vector.reciprocal(out=rp, in_=pv_ps[32 * i:32 * i + 1, :])
                rpb = epp.tile([9, HW], f32, tag="rpb")
                nc.gpsimd.partition_broadcast(rpb[0:9, :], rp[0:1, :])
                nc.vector.tensor_mul(attn_n[32 * i:32 * i + 9, :],
                                     pv_ps[32 * i:32 * i + 9, :], rpb[0:9, :])

            def emit_oat(i, stop):
                for qc in range(2):
                    nc.tensor.matmul(
                        oat_ps[0:64, 512 * qc:512 * qc + 512],
                        wattn_sb[32 * i:32 * i + 9, :],
                        attn_n[32 * i:32 * i + 9, 512 * qc:512 * qc + 512],
                        start=(i == 0), stop=stop,
                        tile_position=(32 * i, 0))

            lg_tiles = {seq[0]: emit_lg(*seq[0])}
            for j, (i, kt) in enumerate(seq):
                if j + 1 < len(seq):
                    lg_tiles[seq[j + 1]] = emit_lg(*seq[j + 1])
                lg_ps = lg_tiles.pop((i, kt))
                pt = ptp.tile([128, HW], bf16, tag="pt")
                nc.scalar.activation(pt, lg_ps[:, :], AF.Exp)
                for qc in range(2):
                    nc.tensor.matmul(
                        pv_ps[32 * i:32 * i + 9, 512 * qc:512 * qc + 512],
                        vt_sb[:, kt, 9 * i:9 * i + 9],
                        pt[:, 512 * qc:512 * qc + 512],
                        start=(kt == 0), stop=(kt == 7),
                        tile_position=(0, 32 * i))
                if kt == 6 and i >= 1:
                    emit_oat(i - 1, stop=False)  # prior head, mul_i surely done
                if kt == 7 and i < 3:
                    emit_norm(i)

            # ---- tail: conv (PE free now) + head-3 epilogue + outputs ----
            conv_ps = ps_lg.tile([64, 512], f32, tag="g")
            nc.tensor.matmul(conv_ps[:, :], wconv_sb, xc_sb)
            emit_norm(3)
            emit_oat(3, stop=True)
            nc.scalar.activation(conv_sb, conv_ps[:, :], AF.Copy)
            nc.sync.dma_start(out=o_conv[:, :], in_=conv_sb)
            nc.scalar.activation(oat_sb[:, 0:512], oat_ps[0:64, 0:512], AF.Copy)
            nc.sync.dma_start(out=o_attn[:, 0:512], in_=oat_sb[:, 0:512])
            nc.vector.tensor_copy(out=oat_sb[:, 512:HW], in_=oat_ps[0:64, 512:HW])
            nc.sync.dma_start(out=o_attn[:, 512:HW], in_=oat_sb[:, 512:HW])

    nc.compile()
    return nc


def _host_prep(inputs):
    import ml_dtypes
    bf = ml_dtypes.bfloat16
    x = np.ascontiguousarray(inputs['x'], np.float32)
    w_qkv = np.ascontiguousarray(inputs['w_qkv'].reshape(2 * DK + DV, C), np.float32)
    b_qkv = np.ascontiguousarray(inputs['b_qkv'], np.float32)
    w_conv = np.ascontiguousarray(inputs['w_conv'].reshape(FILTERS - DV, C), np.float32)
    b_conv = np.ascontiguousarray(inputs['b_conv'], np.float32)
    w_attn = np.ascontiguousarray(inputs['w_attn'].reshape(DV, DV), np.float32)
    b_attn = np.ascontiguousarray(inputs['b_attn'], np.float32)
    rel_h = np.ascontiguousarray(inputs['key_rel_h'], np.float32)  # [63, 8]
    rel_w = np.ascontiguousarray(inputs['key_rel_w'], np.float32)  # [63, 8]

    kk = np.arange(HW)
    DCmat = np.zeros((64, HW), np.float32)
    DCmat[:32] = (kk[None, :] % 32 == np.arange(32)[:, None])
    DCmat[32:] = (kk[None, :] // 32 == np.arange(32)[:, None])
    DCmat = DCmat.astype(bf)

    wconv_aug = np.concatenate([w_conv, b_conv[:, None]], 1).T  # [65, 64]

    in_maps = []
    for c in range(N_CORES):
        b, g = c // 2, c % 2
        heads = [4 * g + i for i in range(4)]
        x_aug = np.concatenate([x[b].reshape(C, HW),
                                np.ones((1, HW), np.float32)], 0)
        wq = w_qkv[32 * g:32 * g + 32] * SCALE
        bq = b_qkv[32 * g:32 * g + 32] * SCALE
        wk = w_qkv[64 + 32 * g:64 + 32 * g + 32]
        bk = b_qkv[64 + 32 * g:64 + 32 * g + 32]
        wqk_aug = np.concatenate(
            [np.concatenate([wq, wk], 0),
             np.concatenate([bq, bk], 0)[:, None]], 1).T  # [65, 64]
        wva_m = np.zeros((65, 36), np.float32)
        wpat_w = np.zeros((65, 4, 63), np.float32)
        wpat_h = np.zeros((65, 4, 63), np.float32)
        for i, h in enumerate(heads):
            wv = w_qkv[128 + 8 * h:128 + 8 * h + 8]
            bv = b_qkv[128 + 8 * h:128 + 8 * h + 8]
            wva_m[64, 9 * i] = 1.0
            wva_m[:64, 9 * i + 1:9 * i + 9] = wv.T
            wva_m[64, 9 * i + 1:9 * i + 9] = bv
            wq_h = w_qkv[8 * h:8 * h + 8] * SCALE
            bq_h = b_qkv[8 * h:8 * h + 8] * SCALE
            wpat_w[:64, i, :] = (rel_w @ wq_h).T
            wpat_w[64, i, :] = rel_w @ bq_h
            wpat_h[:64, i, :] = (rel_h @ wq_h).T
            wpat_h[64, i, :] = rel_h @ bq_h
        wattn_aug = np.zeros((128, 64), np.float32)
        for i, h in enumerate(heads):
            wattn_aug[32 * i + 1:32 * i + 9] = w_attn[:, 8 * h:8 * h + 8].T
        if g == 0:
            wattn_aug[0] += b_attn
        wcat = np.concatenate(
            [wqk_aug, wva_m, wpat_w.reshape(65, 252),
             wpat_h.reshape(65, 252), wconv_aug], 1)  # [65, 668]
        in_maps.append({
            'xbf': np.ascontiguousarray(x_aug.astype(bf)),
            'xc': np.ascontiguousarray(
                x_aug[:, 512 * g:512 * g + 512].astype(bf)),
            'wcat': np.ascontiguousarray(wcat.astype(bf)),
            'wtail': np.ascontiguousarray(wattn_aug.astype(bf)),
            'dconst': DCmat,
        })
    return in_maps


_CACHED = {}


def kernel(**inputs):
    from concourse.bass_utils import run_bass_kernel_spmd
    if 'nc' not in _CACHED:
        _CACHED['nc'] = _build_bass()
    nc = _CACHED['nc']
    in_maps = _host_prep(inputs)
    res = run_bass_kernel_spmd(nc, in_maps, core_ids=list(range(N_CORES)))
    out = np.zeros((B, FILTERS, HW), np.float32)
    for c in range(N_CORES):
        b, g = c // 2, c % 2
        out[b, :64, 512 * g:512 * g + 512] = res.results[c]['o_conv']
        out[b, 64:] += res.results[c]['o_attn']
    return out.reshape(B, FILTERS, H, W)


# revision 11
# speedup vs baseline: 1.0385x; 1.0293x over previous
"""Attention-Augmented Conv2D fused Bass kernel for 8 trn2 NeuronCores (v3).

Problem (hardcoded): x [4,64,32,32], NH=8, DK=DV=64, FILTERS=128 -> out [4,128,32,32].
Sharding: core c -> batch b=c//2, head-group g=c%2 (heads 4g..4g+4).
Each core produces:
  o_conv [64,512]  : conv1x1 output for its batch, positions [512g, 512g+512)
  o_attn [64,1024] : partial attn-out conv over its 4 heads (bias only on g==0)
Host gather: conv halves concatenated, attn partials summed per batch.

All projections fold their bias via a ones-row appended to x (x_aug [65,1024],
uploaded in bf16). Relative-position logits fold into the single logits matmul
with K-dim 72:
    KA_i = [D_w (32, k%32 indicator) ; D_h (32, k//32 indicator) ; K_i (8)]
    QA_i = [patwT (32) ; pathT (32) ; Q_i (8)]
patwT/pathT come from per-head rel-projections pf_w = (rel_w@Wq)^T@x_aug (on the
column-permuted q' = 32y+u index) and pf_h = (rel_h@Wq)^T@x_aug, DMA'd straight
from PSUM to a DRAM scratch (f32) and gathered back with a shifted (Toeplitz)
access pattern; a DVE copy converts/unscrambles into the bf16 QA tiles.

Softmax skips max-subtraction (logits are O(few)); the denominator comes from a
ones-column in the V projection, so the PV matmul also produces the softmax
denominator (pv row 32i per head). The epilogue is per-head and overlaps the
next head's main loop: reciprocal (DVE) -> partition_broadcast (GPSIMD) ->
pv*rp (DVE, bf16); the wattn^T @ attn_n accumulation is deferred into the PE
stream at the next head's kt=6 so PE never stalls on the DVE chain. Heads 2-3's
projections are interleaved into the head-0 loop iterations. A short PE warmup
at t~0 starts the p-state ramp so the real projections run at full clock.
"""
import sys
import numpy as np

sys.path.insert(0, '/opt/trn_rl_repo')

NH, DK, DV, FILTERS = 8, 64, 64, 128
B, C, H, W = 4, 64, 32, 32
HW = H * W
dkh = DK // NH
SCALE = dkh ** -0.5
N_CORES = 8


def _build_bass():
    import concourse.bass as bass
    import concourse.bacc as bacc
    import concourse.mybir as mybir
    import concourse.tile as tile

    f32 = mybir.dt.float32
    bf16 = mybir.dt.bfloat16
    AF = mybir.ActivationFunctionType

    nc = bacc.Bacc()

    xbf = nc.dram_tensor("xbf", [65, HW], bf16, kind="ExternalInput")
    xc = nc.dram_tensor("xc", [65, 512], bf16, kind="ExternalInput")
    wcat = nc.dram_tensor("wcat", [65, 668], bf16, kind="ExternalInput")
    wtail = nc.dram_tensor("wtail", [128, 64], bf16, kind="ExternalInput")
    dconst = nc.dram_tensor("dconst", [64, HW], bf16, kind="ExternalInput")
    o_conv = nc.dram_tensor("o_conv", [64, 512], f32, kind="ExternalOutput")
    o_attn = nc.dram_tensor("o_attn", [64, HW], f32, kind="ExternalOutput")
    scr_w = nc.dram_tensor("scr_w", [4, 64, HW], bf16)
    scr_h = nc.dram_tensor("scr_h", [4, 64, HW], bf16)

    with tile.TileContext(nc) as tc:
        with (
            tc.tile_pool(name="const", bufs=1) as constp,
            tc.tile_pool(name="kaqa", bufs=4) as kaqap,
            tc.tile_pool(name="pf", bufs=3) as pfp,
            tc.tile_pool(name="tmp", bufs=2) as tmpp,
            tc.tile_pool(name="pt", bufs=3) as ptp,
            tc.tile_pool(name="ep", bufs=2) as epp,
            tc.tile_pool(name="outp", bufs=1) as outp,
            tc.tile_pool(name="ps_lg", bufs=2, space="PSUM") as ps_lg,
            tc.tile_pool(name="ps_pf", bufs=1, space="PSUM") as ps_pf,
            tc.tile_pool(name="ps_pv", bufs=1, space="PSUM") as ps_pv,
        ):
            # ---- input DMAs ----
            x_sb = constp.tile([65, HW], bf16, tag="x")
            for qc in range(2):
                nc.sync.dma_start(out=x_sb[:, 512 * qc:512 * qc + 512],
                                  in_=xbf[:, 512 * qc:512 * qc + 512])
            wcat_sb = constp.tile([65, 668], bf16, tag="wcat")
            nc.scalar.dma_start(out=wcat_sb, in_=wcat[:, :])
            xc_sb = constp.tile([65, 512], bf16, tag="xc")
            wqk_sb = wcat_sb[:, 0:64]
            wva_sb = wcat_sb[:, 64:100]
            wpatw_sb = wcat_sb[:, 100:352].rearrange("c (h m) -> c h m", h=4)
            wpath_sb = wcat_sb[:, 352:604].rearrange("c (h m) -> c h m", h=4)
            wconv_sb = wcat_sb[:, 604:668]
            wattn_sb = constp.tile([128, 64], bf16, tag="wtail")

            ka, qa = [], []
            for i in range(4):
                ka_i = kaqap.tile([72, HW], bf16, tag="ka", name=f"ka{i}")
                qa_i = kaqap.tile([72, HW], bf16, tag="qa", name=f"qa{i}")
                ka.append(ka_i)
                qa.append(qa_i)
            nc.gpsimd.dma_start(out=ka[0][0:64, :], in_=dconst[:, :])

            # ---- PE warmup: start the p-state ramp clock early ----
            warm_s = constp.tile([1, 16], f32, tag="warms")
            warm_m = constp.tile([1, 128], f32, tag="warmm")
            nc.vector.memset(warm_s, 0.0)
            nc.vector.memset(warm_m, 0.0)
            warm_ps = ps_lg.tile([16, 128], f32, tag="g")
            for _ in range(4):
                nc.tensor.matmul(warm_ps[:, :], warm_s[:, 0:16], warm_m[:, :])
            # preload the activation table while Act waits for inputs
            warm_a = constp.tile([1, 16], f32, tag="warma")
            nc.scalar.activation(warm_a, warm_s[:, :], AF.Exp)

            # ---- qk projection ----
            qk_ps = ps_pv.tile([64, HW], f32, tag="v")
            for qc in range(2):
                nc.tensor.matmul(qk_ps[:, 512 * qc:512 * qc + 512],
                                 wqk_sb, x_sb[:, 512 * qc:512 * qc + 512])

            xp = x_sb.rearrange("c (u y) -> c y u", y=32)
            pf_pss, pfw_sbs, pfh_sbs, tms = [], [], [], []

            def emit_pf_mms(i):
                # rows 0..63 = pf_w on q' (permuted) columns, 64..127 = pf_h
                pf_ps = ps_pf.tile([127, HW], f32, tag="f", name=f"pf{i}")
                pf_pss.append(pf_ps)
                for qc in range(2):
                    nc.tensor.matmul(pf_ps[0:63, 512 * qc:512 * qc + 512],
                                     wpatw_sb[:, i, :],
                                     xp[:, 16 * qc:16 * qc + 16, :])
                for qc in range(2):
                    nc.tensor.matmul(pf_ps[64:127, 512 * qc:512 * qc + 512],
                                     wpath_sb[:, i, :],
                                     x_sb[:, 512 * qc:512 * qc + 512],
                                     tile_position=(0, 64))

            def emit_pf_copies(i, w_eng):
                pfw_sb = pfp.tile([63, HW], bf16, tag="pf", name=f"pfw{i}")
                pfh_sb = pfp.tile([63, HW], bf16, tag="pf", name=f"pfh{i}")
                pfw_sbs.append(pfw_sb)
                pfh_sbs.append(pfh_sb)
                if w_eng == 'act':
                    nc.scalar.activation(pfw_sb, pf_pss[i][0:63, :], AF.Copy)
                else:
                    nc.vector.tensor_copy(out=pfw_sb, in_=pf_pss[i][0:63, :])
                nc.vector.tensor_copy(out=pfh_sb, in_=pf_pss[i][64:127, :])

            def emit_scratch(i):
                # scratch writes + shifted gathers (SP HWDGE queue)
                nc.sync.dma_start(out=scr_w[i, 0:63, :], in_=pfw_sbs[i])
                nc.sync.dma_start(out=scr_h[i, 0:63, :], in_=pfh_sbs[i])
                tm = tmpp.tile([32, HW], bf16, tag="tm", name=f"tm{i}")
                tms.append(tm)
                # patwT[wk, (y,u)'] = pf_w[31+wk-y, .]: flat 31744+1024wk-992y+u
                nc.sync.dma_start(
                    out=tm.rearrange("p (y u) -> p y u", y=32),
                    in_=bass.AP(scr_w, i * 64 * HW + 31744,
                                [[1024, 32], [-992, 32], [1, 32]]))
                # pathT[hk, 32u+y] = pf_h[31+hk-u, q]: flat 31744+1024hk-992u+y
                nc.sync.dma_start(
                    out=qa[i][32:64, :].rearrange("p (u y) -> p u y", y=32),
                    in_=bass.AP(scr_h, i * 64 * HW + 31744,
                                [[1024, 32], [-992, 32], [1, 32]]))

            def emit_unscr(i):
                # unscramble q' -> q on DVE: qa[wk, 32u+y] = tm[wk, 32y+u]
                nc.vector.tensor_copy(
                    out=qa[i][0:32, :].rearrange("p (u y) -> p u y", y=32),
                    in_=tms[i].rearrange("p (y u) -> p u y", u=32))

            def emit_qk_rows(i):
                nc.gpsimd.dma_start(out=ka[i][64:72, :],
                                    in_=qk_bf[32 + 8 * i:40 + 8 * i, :])
                nc.gpsimd.dma_start(out=qa[i][64:72, :],
                                    in_=qk_bf[8 * i:8 * i + 8, :])

            # head 0 critical chain first
            emit_pf_mms(0)
            emit_pf_copies(0, 'act')
            emit_scratch(0)
            emit_unscr(0)
            qk_bf = constp.tile([64, HW], bf16, tag="qk")
            nc.scalar.activation(qk_bf, qk_ps[:, :], AF.Copy)
            emit_qk_rows(0)
            # head 1
            emit_pf_mms(1)
            emit_pf_copies(1, 'act')
            emit_scratch(1)
            emit_unscr(1)
            nc.gpsimd.dma_start(out=ka[1][0:64, :], in_=ka[0][0:64, :])
            emit_qk_rows(1)
            # V^T projection (needed at pv(0,0), shortly after lg(0,0))
            vt_ps = ps_lg.tile([128, 8, 36], f32, tag="g")
            for kt in range(8):
                nc.tensor.matmul(vt_ps[:, kt, :],
                                 x_sb[:, 128 * kt:128 * kt + 128], wva_sb)
            vt_sb = constp.tile([128, 8, 36], bf16, tag="vt")
            nc.scalar.activation(vt_sb, vt_ps[:, :, :], AF.Copy)
            nc.gpsimd.dma_start(out=wattn_sb, in_=wtail[:, :])

            # ---- main attention loop (heads 2-3 projections interleaved) ----
            pv_ps = ps_pv.tile([128, HW], f32, tag="v")
            attn_n = outp.tile([128, HW], bf16, tag="attn")
            oat_sb = outp.tile([64, HW], f32, tag="oat")
            conv_sb = outp.tile([64, 512], f32, tag="oconv")
            oat_ps = None
            seq = [(i, kt) for i in range(4) for kt in range(8)]

            def emit_lg(i, kt):
                lg_ps = ps_lg.tile([128, HW], f32, tag="g")
                for qc in range(2):
                    nc.tensor.matmul(
                        lg_ps[:, 512 * qc:512 * qc + 512],
                        ka[i][:, 128 * kt:128 * kt + 128],
                        qa[i][:, 512 * qc:512 * qc + 512])
                return lg_ps

            def emit_norm(i):
                # DVE/GPSIMD only -- never blocks the PE stream
                rp = epp.tile([1, HW], f32, tag="rp")
                nc.vector.reciprocal(out=rp, in_=pv_ps[32 * i:32 * i + 1, :])
                rpb = epp.tile([9, HW], f32, tag="rpb")
                nc.gpsimd.partition_broadcast(rpb[0:9, :], rp[0:1, :])
                nc.vector.tensor_mul(attn_n[32 * i:32 * i + 9, :],
                                     pv_ps[32 * i:32 * i + 9, :], rpb[0:9, :])

            def emit_oat(i, stop):
                for qc in range(2):
                    nc.tensor.matmul(
                        oat_ps[0:64, 512 * qc:512 * qc + 512],
                        wattn_sb[32 * i:32 * i + 9, :],
                        attn_n[32 * i:32 * i + 9, 512 * qc:512 * qc + 512],
                        start=(i == 0), stop=stop,
                        tile_position=(32 * i, 0))

            lg_tiles = {seq[0]: emit_lg(*seq[0])}
            for j, (i, kt) in enumerate(seq):
                if j + 1 < len(seq):
                    lg_tiles[seq[j + 1]] = emit_lg(*seq[j + 1])
                lg_ps = lg_tiles.pop((i, kt))
                pt = ptp.tile([128, HW], bf16, tag="pt")
                nc.scalar.activation(pt, lg_ps[:, :], AF.Exp)
                for qc in range(2):
                    nc.tensor.matmul(
                        pv_ps[32 * i:32 * i + 9, 512 * qc:512 * qc + 512],
                        vt_sb[:, kt, 9 * i:9 * i + 9],
                        pt[:, 512 * qc:512 * qc + 512],
                        start=(kt == 0), stop=(kt == 7),
                        tile_position=(0, 32 * i))
                if (i, kt) == (0, 3):
                    emit_pf_mms(2)
                    emit_pf_copies(2, 'dve')
                    emit_scratch(2)
                    emit_unscr(2)
                    nc.gpsimd.dma_start(out=ka[2][0:64, :], in_=ka[0][0:64, :])
                    emit_qk_rows(2)
                if (i, kt) == (0, 6):
                    emit_pf_mms(3)
                    emit_pf_copies(3, 'dve')
                    emit_scratch(3)
                    nc.gpsimd.dma_start(out=ka[3][0:64, :], in_=ka[0][0:64, :])
                    emit_qk_rows(3)
                if (i, kt) == (0, 7):
                    oat_ps = ps_pf.tile([64, HW], f32, tag="f")
                if kt == 7 and i < 3:
                    emit_norm(i)
                    if i == 0:
                        emit_unscr(3)  # DVE: after head-0's norm chain
                if kt == 6 and i >= 1:
                    emit_oat(i - 1, stop=False)  # prior head, mul_i done by now
                if (i, kt) == (1, 6):
                    nc.gpsimd.dma_start(out=xc_sb, in_=xc[:, :])

            # ---- tail: conv (PE free now) + head-3 epilogue + outputs ----
            conv_ps = ps_lg.tile([64, 512], f32, tag="g")
            nc.tensor.matmul(conv_ps[:, :], wconv_sb, xc_sb)
            emit_norm(3)
            emit_oat(3, stop=True)
            nc.scalar.activation(conv_sb, conv_ps[:, :], AF.Copy)
            nc.sync.dma_start(out=o_conv[:, :], in_=conv_sb)
            nc.scalar.activation(oat_sb[:, 0:512], oat_ps[0:64, 0:512], AF.Copy)
            nc.sync.dma_start(out=o_attn[:, 0:512], in_=oat_sb[:, 0:512])
            nc.vector.tensor_copy(out=oat_sb[:, 512:HW], in_=oat_ps[0:64, 512:HW])
            nc.sync.dma_start(out=o_attn[:, 512:HW], in_=oat_sb[:, 512:HW])

    nc.compile()
    return nc


def _host_prep(inputs):
    import ml_dtypes
    bf = ml_dtypes.bfloat16
    x = np.ascontiguousarray(inputs['x'], np.float32)
    w_qkv = np.ascontiguousarray(inputs['w_qkv'].reshape(2 * DK + DV, C), np.float32)
    b_qkv = np.ascontiguousarray(inputs['b_qkv'], np.float32)
    w_conv = np.ascontiguousarray(inputs['w_conv'].reshape(FILTERS - DV, C), np.float32)
    b_conv = np.ascontiguousarray(inputs['b_conv'], np.float32)
    w_attn = np.ascontiguousarray(inputs['w_attn'].reshape(DV, DV), np.float32)
    b_attn = np.ascontiguousarray(inputs['b_attn'], np.float32)
    rel_h = np.ascontiguousarray(inputs['key_rel_h'], np.float32)  # [63, 8]
    rel_w = np.ascontiguousarray(inputs['key_rel_w'], np.float32)  # [63, 8]

    kk = np.arange(HW)
    DCmat = np.zeros((64, HW), np.float32)
    DCmat[:32] = (kk[None, :] % 32 == np.arange(32)[:, None])
    DCmat[32:] = (kk[None, :] // 32 == np.arange(32)[:, None])
    DCmat = DCmat.astype(bf)

    wconv_aug = np.concatenate([w_conv, b_conv[:, None]], 1).T  # [65, 64]

    in_maps = []
    for c in range(N_CORES):
        b, g = c // 2, c % 2
        heads = [4 * g + i for i in range(4)]
        x_aug = np.concatenate([x[b].reshape(C, HW),
                                np.ones((1, HW), np.float32)], 0)
        wq = w_qkv[32 * g:32 * g + 32] * SCALE
        bq = b_qkv[32 * g:32 * g + 32] * SCALE
        wk = w_qkv[64 + 32 * g:64 + 32 * g + 32]
        bk = b_qkv[64 + 32 * g:64 + 32 * g + 32]
        wqk_aug = np.concatenate(
            [np.concatenate([wq, wk], 0),
             np.concatenate([bq, bk], 0)[:, None]], 1).T  # [65, 64]
        wva_m = np.zeros((65, 36), np.float32)
        wpat_w = np.zeros((65, 4, 63), np.float32)
        wpat_h = np.zeros((65, 4, 63), np.float32)
        for i, h in enumerate(heads):
            wv = w_qkv[128 + 8 * h:128 + 8 * h + 8]
            bv = b_qkv[128 + 8 * h:128 + 8 * h + 8]
            wva_m[64, 9 * i] = 1.0
            wva_m[:64, 9 * i + 1:9 * i + 9] = wv.T
            wva_m[64, 9 * i + 1:9 * i + 9] = bv
            wq_h = w_qkv[8 * h:8 * h + 8] * SCALE
            bq_h = b_qkv[8 * h:8 * h + 8] * SCALE
            wpat_w[:64, i, :] = (rel_w @ wq_h).T
            wpat_w[64, i, :] = rel_w @ bq_h
            wpat_h[:64, i, :] = (rel_h @ wq_h).T
            wpat_h[64, i, :] = rel_h @ bq_h
        wattn_aug = np.zeros((128, 64), np.float32)
        for i, h in enumerate(heads):
            wattn_aug[32 * i + 1:32 * i + 9] = w_attn[:, 8 * h:8 * h + 8].T
        if g == 0:
            wattn_aug[0] += b_attn
        wcat = np.concatenate(
            [wqk_aug, wva_m, wpat_w.reshape(65, 252),
             wpat_h.reshape(65, 252), wconv_aug], 1)  # [65, 668]
        in_maps.append({
            'xbf': np.ascontiguousarray(x_aug.astype(bf)),
            'xc': np.ascontiguousarray(
                x_aug[:, 512 * g:512 * g + 512].astype(bf)),
            'wcat': np.ascontiguousarray(wcat.astype(bf)),
            'wtail': np.ascontiguousarray(wattn_aug.astype(bf)),
            'dconst': DCmat,
        })
    return in_maps


_CACHED = {}


def kernel(**inputs):
    from concourse.bass_utils import run_bass_kernel_spmd
    if 'nc' not in _CACHED:
        _CACHED['nc'] = _build_bass()
    nc = _CACHED['nc']
    in_maps = _host_prep(inputs)
    res = run_bass_kernel_spmd(nc, in_maps, core_ids=list(range(N_CORES)))
    out = np.zeros((B, FILTERS, HW), np.float32)
    for c in range(N_CORES):
        b, g = c // 2, c % 2
        out[b, :64, 512 * g:512 * g + 512] = res.results[c]['o_conv']
        out[b, 64:] += res.results[c]['o_attn']
    return out.reshape(B, FILTERS, H, W)


# revision 12
# speedup vs baseline: 1.0445x; 1.0058x over previous
"""Attention-Augmented Conv2D fused Bass kernel for 8 trn2 NeuronCores (v3).

Problem (hardcoded): x [4,64,32,32], NH=8, DK=DV=64, FILTERS=128 -> out [4,128,32,32].
Sharding: core c -> batch b=c//2, head-group g=c%2 (heads 4g..4g+4).
Each core produces:
  o_conv [64,512]  : conv1x1 output for its batch, positions [512g, 512g+512)
  o_attn [64,1024] : partial attn-out conv over its 4 heads (bias only on g==0)
Host gather: conv halves concatenated, attn partials summed per batch.

All projections fold their bias via a ones-row appended to x (x_aug [65,1024],
uploaded in bf16). Relative-position logits fold into the single logits matmul
with K-dim 72:
    KA_i = [D_w (32, k%32 indicator) ; D_h (32, k//32 indicator) ; K_i (8)]
    QA_i = [patwT (32) ; pathT (32) ; Q_i (8)]
patwT/pathT come from per-head rel-projections pf_w = (rel_w@Wq)^T@x_aug (on the
column-permuted q' = 32y+u index) and pf_h = (rel_h@Wq)^T@x_aug, DMA'd straight
from PSUM to a DRAM scratch (f32) and gathered back with a shifted (Toeplitz)
access pattern; a DVE copy converts/unscrambles into the bf16 QA tiles.

Softmax skips max-subtraction (logits are O(few)); the denominator comes from a
ones-column in the V projection, so the PV matmul also produces the softmax
denominator (pv row 32i per head). The epilogue is per-head and overlaps the
next head's main loop: reciprocal (DVE) -> partition_broadcast (GPSIMD) ->
pv*rp (DVE, bf16); the wattn^T @ attn_n accumulation is deferred into the PE
stream at the next head's kt=6 so PE never stalls on the DVE chain. Heads 2-3's
projections are interleaved into the head-0 loop iterations. A short PE warmup
at t~0 starts the p-state ramp so the real projections run at full clock.
"""
import sys
import numpy as np

sys.path.insert(0, '/opt/trn_rl_repo')

NH, DK, DV, FILTERS = 8, 64, 64, 128
B, C, H, W = 4, 64, 32, 32
HW = H * W
dkh = DK // NH
SCALE = dkh ** -0.5
N_CORES = 8


def _build_bass():
    import concourse.bass as bass
    import concourse.bacc as bacc
    import concourse.mybir as mybir
    import concourse.tile as tile

    f32 = mybir.dt.float32
    bf16 = mybir.dt.bfloat16
    AF = mybir.ActivationFunctionType

    nc = bacc.Bacc()

    xbf = nc.dram_tensor("xbf", [65, HW], bf16, kind="ExternalInput")
    xc = nc.dram_tensor("xc", [65, 512], bf16, kind="ExternalInput")
    wcat = nc.dram_tensor("wcat", [65, 668], bf16, kind="ExternalInput")
    wtail = nc.dram_tensor("wtail", [128, 64], bf16, kind="ExternalInput")
    dconst = nc.dram_tensor("dconst", [64, HW], bf16, kind="ExternalInput")
    o_conv = nc.dram_tensor("o_conv", [64, 512], f32, kind="ExternalOutput")
    o_attn = nc.dram_tensor("o_attn", [64, HW], f32, kind="ExternalOutput")
    scr_w = nc.dram_tensor("scr_w", [4, 64, HW], bf16)
    scr_h = nc.dram_tensor("scr_h", [4, 64, HW], bf16)

    with tile.TileContext(nc) as tc:
        with (
            tc.tile_pool(name="const", bufs=1) as constp,
            tc.tile_pool(name="kaqa", bufs=4) as kaqap,
            tc.tile_pool(name="pf", bufs=3) as pfp,
            tc.tile_pool(name="tmp", bufs=2) as tmpp,
            tc.tile_pool(name="pt", bufs=3) as ptp,
            tc.tile_pool(name="ep", bufs=2) as epp,
            tc.tile_pool(name="outp", bufs=1) as outp,
            tc.tile_pool(name="ps_lg", bufs=2, space="PSUM") as ps_lg,
            tc.tile_pool(name="ps_pf", bufs=1, space="PSUM") as ps_pf,
            tc.tile_pool(name="ps_pv", bufs=1, space="PSUM") as ps_pv,
        ):
            # ---- input DMAs ----
            x_sb = constp.tile([65, HW], bf16, tag="x")
            for qc in range(2):
                nc.sync.dma_start(out=x_sb[:, 512 * qc:512 * qc + 512],
                                  in_=xbf[:, 512 * qc:512 * qc + 512])
            wcat_sb = constp.tile([65, 668], bf16, tag="wcat")
            nc.scalar.dma_start(out=wcat_sb, in_=wcat[:, :])
            xc_sb = constp.tile([65, 512], bf16, tag="xc")
            wqk_sb = wcat_sb[:, 0:64]
            wva_sb = wcat_sb[:, 64:100]
            wpatw_sb = wcat_sb[:, 100:352].rearrange("c (h m) -> c h m", h=4)
            wpath_sb = wcat_sb[:, 352:604].rearrange("c (h m) -> c h m", h=4)
            wconv_sb = wcat_sb[:, 604:668]
            wattn_sb = constp.tile([128, 64], bf16, tag="wtail")

            ka, qa = [], []
            for i in range(4):
                ka_i = kaqap.tile([72, HW], bf16, tag="ka", name=f"ka{i}")
                qa_i = kaqap.tile([72, HW], bf16, tag="qa", name=f"qa{i}")
                ka.append(ka_i)
                qa.append(qa_i)
            nc.gpsimd.dma_start(out=ka[0][0:64, :], in_=dconst[:, :])

            # ---- PE warmup: start the p-state ramp clock early ----
            warm_s = constp.tile([1, 16], f32, tag="warms")
            warm_m = constp.tile([1, 128], f32, tag="warmm")
            nc.vector.memset(warm_s, 0.0)
            nc.vector.memset(warm_m, 0.0)
            warm_ps = ps_lg.tile([16, 128], f32, tag="g")
            for _ in range(4):
                nc.tensor.matmul(warm_ps[:, :], warm_s[:, 0:16], warm_m[:, :])
            # preload the activation table while Act waits for inputs
            warm_a = constp.tile([1, 16], f32, tag="warma")
            nc.scalar.activation(warm_a, warm_s[:, :], AF.Exp)

            # ---- qk projection ----
            qk_ps = ps_pv.tile([64, HW], f32, tag="v")
            for qc in range(2):
                nc.tensor.matmul(qk_ps[:, 512 * qc:512 * qc + 512],
                                 wqk_sb, x_sb[:, 512 * qc:512 * qc + 512])

            xp = x_sb.rearrange("c (u y) -> c y u", y=32)
            pf_pss, pfw_sbs, pfh_sbs, tms = [], [], [], []

            def emit_pf_mms(i):
                # rows 0..63 = pf_w on q' (permuted) columns, 64..127 = pf_h
                pf_ps = ps_pf.tile([127, HW], f32, tag="f", name=f"pf{i}")
                pf_pss.append(pf_ps)
                for qc in range(2):
                    nc.tensor.matmul(pf_ps[0:63, 512 * qc:512 * qc + 512],
                                     wpatw_sb[:, i, :],
                                     xp[:, 16 * qc:16 * qc + 16, :])
                for qc in range(2):
                    nc.tensor.matmul(pf_ps[64:127, 512 * qc:512 * qc + 512],
                                     wpath_sb[:, i, :],
                                     x_sb[:, 512 * qc:512 * qc + 512],
                                     tile_position=(0, 64))

            def emit_pf_copies(i, eng):
                pf_sb = pfp.tile([127, HW], bf16, tag="pf", name=f"pfc{i}")
                pfw_sbs.append(pf_sb)
                if eng == 'act':
                    nc.scalar.activation(pf_sb, pf_pss[i][:, :], AF.Copy)
                else:
                    nc.vector.tensor_copy(out=pf_sb, in_=pf_pss[i][:, :])

            def emit_scratch(i):
                # scratch writes + shifted gathers (SP HWDGE queue)
                nc.sync.dma_start(out=scr_w[i, 0:63, :], in_=pfw_sbs[i][0:63, :])
                nc.sync.dma_start(out=scr_h[i, 0:63, :], in_=pfw_sbs[i][64:127, :])
                tm = tmpp.tile([32, HW], bf16, tag="tm", name=f"tm{i}")
                tms.append(tm)
                # patwT[wk, (y,u)'] = pf_w[31+wk-y, .]: flat 31744+1024wk-992y+u
                nc.sync.dma_start(
                    out=tm.rearrange("p (y u) -> p y u", y=32),
                    in_=bass.AP(scr_w, i * 64 * HW + 31744,
                                [[1024, 32], [-992, 32], [1, 32]]))
                # pathT[hk, 32u+y] = pf_h[31+hk-u, q]: flat 31744+1024hk-992u+y
                nc.sync.dma_start(
                    out=qa[i][32:64, :].rearrange("p (u y) -> p u y", y=32),
                    in_=bass.AP(scr_h, i * 64 * HW + 31744,
                                [[1024, 32], [-992, 32], [1, 32]]))

            def emit_unscr(i):
                # unscramble q' -> q on DVE: qa[wk, 32u+y] = tm[wk, 32y+u]
                nc.vector.tensor_copy(
                    out=qa[i][0:32, :].rearrange("p (u y) -> p u y", y=32),
                    in_=tms[i].rearrange("p (y u) -> p u y", u=32))

            def emit_qk_rows(i):
                nc.gpsimd.dma_start(out=ka[i][64:72, :],
                                    in_=qk_bf[32 + 8 * i:40 + 8 * i, :])
                nc.gpsimd.dma_start(out=qa[i][64:72, :],
                                    in_=qk_bf[8 * i:8 * i + 8, :])

            # qkc early on Act, then head-0 critical chain
            qk_bf = constp.tile([64, HW], bf16, tag="qk")
            nc.scalar.activation(qk_bf, qk_ps[:, :], AF.Copy)
            emit_qk_rows(0)
            emit_pf_mms(0)
            emit_pf_copies(0, 'act')
            emit_scratch(0)
            emit_unscr(0)
            # head 1
            emit_qk_rows(1)
            nc.gpsimd.dma_start(out=ka[1][0:64, :], in_=ka[0][0:64, :])
            emit_pf_mms(1)
            emit_pf_copies(1, 'act')
            emit_scratch(1)
            emit_unscr(1)
            # V^T projection (needed at pv(0,0), shortly after lg(0,0))
            vt_ps = ps_lg.tile([128, 8, 36], f32, tag="g")
            for kt in range(8):
                nc.tensor.matmul(vt_ps[:, kt, :],
                                 x_sb[:, 128 * kt:128 * kt + 128], wva_sb)
            vt_sb = constp.tile([128, 8, 36], bf16, tag="vt")
            nc.scalar.activation(vt_sb, vt_ps[:, :, :], AF.Copy)
            nc.gpsimd.dma_start(out=wattn_sb, in_=wtail[:, :])

            # ---- main attention loop (heads 2-3 projections interleaved) ----
            pv_ps = ps_pv.tile([128, HW], f32, tag="v")
            attn_n = outp.tile([128, HW], bf16, tag="attn")
            oat_sb = outp.tile([64, HW], f32, tag="oat")
            conv_sb = outp.tile([64, 512], f32, tag="oconv")
            oat_ps = None
            seq = [(i, kt) for i in range(4) for kt in range(8)]

            def emit_lg(i, kt):
                lg_ps = ps_lg.tile([128, HW], f32, tag="g")
                for qc in range(2):
                    nc.tensor.matmul(
                        lg_ps[:, 512 * qc:512 * qc + 512],
                        ka[i][:, 128 * kt:128 * kt + 128],
                        qa[i][:, 512 * qc:512 * qc + 512])
                return lg_ps

            def emit_norm(i):
                # DVE/GPSIMD only -- never blocks the PE stream
                rp = epp.tile([1, HW], f32, tag="rp")
                nc.vector.reciprocal(out=rp, in_=pv_ps[32 * i:32 * i + 1, :])
                rpb = epp.tile([9, HW], f32, tag="rpb")
                nc.gpsimd.partition_broadcast(rpb[0:9, :], rp[0:1, :])
                nc.vector.tensor_mul(attn_n[32 * i:32 * i + 9, :],
                                     pv_ps[32 * i:32 * i + 9, :], rpb[0:9, :])

            def emit_oat(i, stop):
                for qc in range(2):
                    nc.tensor.matmul(
                        oat_ps[0:64, 512 * qc:512 * qc + 512],
                        wattn_sb[32 * i:32 * i + 9, :],
                        attn_n[32 * i:32 * i + 9, 512 * qc:512 * qc + 512],
                        start=(i == 0), stop=stop,
                        tile_position=(32 * i, 0))

            lg_tiles = {seq[0]: emit_lg(*seq[0])}
            for j, (i, kt) in enumerate(seq):
                if j + 1 < len(seq):
                    lg_tiles[seq[j + 1]] = emit_lg(*seq[j + 1])
                lg_ps = lg_tiles.pop((i, kt))
                pt = ptp.tile([128, HW], bf16, tag="pt")
                nc.scalar.activation(pt, lg_ps[:, :], AF.Exp)
                for qc in range(2):
                    nc.tensor.matmul(
                        pv_ps[32 * i:32 * i + 9, 512 * qc:512 * qc + 512],
                        vt_sb[:, kt, 9 * i:9 * i + 9],
                        pt[:, 512 * qc:512 * qc + 512],
                        start=(kt == 0), stop=(kt == 7),
                        tile_position=(0, 32 * i))
                if (i, kt) == (0, 3):
                    emit_pf_mms(2)
                    emit_pf_copies(2, 'dve')
                    emit_scratch(2)
                    emit_unscr(2)
                    nc.gpsimd.dma_start(out=ka[2][0:64, :], in_=ka[0][0:64, :])
                    emit_qk_rows(2)
                if (i, kt) == (0, 6):
                    emit_pf_mms(3)
                    emit_pf_copies(3, 'dve')
                    emit_scratch(3)
                    nc.gpsimd.dma_start(out=ka[3][0:64, :], in_=ka[0][0:64, :])
                    emit_qk_rows(3)
                if (i, kt) == (0, 7):
                    oat_ps = ps_pf.tile([64, HW], f32, tag="f")
                if kt == 7 and i >= 1:
                    emit_oat(i - 1, stop=False)  # prior head, mul_i done by now
                if kt == 7 and i < 3:
                    emit_norm(i)
                    if i == 0:
                        emit_unscr(3)  # DVE: after head-0's norm chain
                if (i, kt) == (1, 6):
                    nc.gpsimd.dma_start(out=xc_sb, in_=xc[:, :])

            # ---- tail: conv (PE free now) + head-3 epilogue + outputs ----
            conv_ps = ps_lg.tile([64, 512], f32, tag="g")
            nc.tensor.matmul(conv_ps[:, :], wconv_sb, xc_sb)
            emit_norm(3)
            emit_oat(3, stop=True)
            nc.scalar.activation(conv_sb, conv_ps[:, :], AF.Copy)
            nc.sync.dma_start(out=o_conv[:, :], in_=conv_sb)
            nc.scalar.activation(oat_sb[:, 0:512], oat_ps[0:64, 0:512], AF.Copy)
            nc.sync.dma_start(out=o_attn[:, 0:512], in_=oat_sb[:, 0:512])
            nc.vector.tensor_copy(out=oat_sb[:, 512:HW], in_=oat_ps[0:64, 512:HW])
            nc.sync.dma_start(out=o_attn[:, 512:HW], in_=oat_sb[:, 512:HW])

    nc.compile()
    return nc


def _host_prep(inputs):
    import ml_dtypes
    bf = ml_dtypes.bfloat16
    x = np.ascontiguousarray(inputs['x'], np.float32)
    w_qkv = np.ascontiguousarray(inputs['w_qkv'].reshape(2 * DK + DV, C), np.float32)
    b_qkv = np.ascontiguousarray(inputs['b_qkv'], np.float32)
    w_conv = np.ascontiguousarray(inputs['w_conv'].reshape(FILTERS - DV, C), np.float32)
    b_conv = np.ascontiguousarray(inputs['b_conv'], np.float32)
    w_attn = np.ascontiguousarray(inputs['w_attn'].reshape(DV, DV), np.float32)
    b_attn = np.ascontiguousarray(inputs['b_attn'], np.float32)
    rel_h = np.ascontiguousarray(inputs['key_rel_h'], np.float32)  # [63, 8]
    rel_w = np.ascontiguousarray(inputs['key_rel_w'], np.float32)  # [63, 8]

    kk = np.arange(HW)
    DCmat = np.zeros((64, HW), np.float32)
    DCmat[:32] = (kk[None, :] % 32 == np.arange(32)[:, None])
    DCmat[32:] = (kk[None, :] // 32 == np.arange(32)[:, None])
    DCmat = DCmat.astype(bf)

    wconv_aug = np.concatenate([w_conv, b_conv[:, None]], 1).T  # [65, 64]

    in_maps = []
    for c in range(N_CORES):
        b, g = c // 2, c % 2
        heads = [4 * g + i for i in range(4)]
        x_aug = np.concatenate([x[b].reshape(C, HW),
                                np.ones((1, HW), np.float32)], 0)
        wq = w_qkv[32 * g:32 * g + 32] * SCALE
        bq = b_qkv[32 * g:32 * g + 32] * SCALE
        wk = w_qkv[64 + 32 * g:64 + 32 * g + 32]
        bk = b_qkv[64 + 32 * g:64 + 32 * g + 32]
        wqk_aug = np.concatenate(
            [np.concatenate([wq, wk], 0),
             np.concatenate([bq, bk], 0)[:, None]], 1).T  # [65, 64]
        wva_m = np.zeros((65, 36), np.float32)
        wpat_w = np.zeros((65, 4, 63), np.float32)
        wpat_h = np.zeros((65, 4, 63), np.float32)
        for i, h in enumerate(heads):
            wv = w_qkv[128 + 8 * h:128 + 8 * h + 8]
            bv = b_qkv[128 + 8 * h:128 + 8 * h + 8]
            wva_m[64, 9 * i] = 1.0
            wva_m[:64, 9 * i + 1:9 * i + 9] = wv.T
            wva_m[64, 9 * i + 1:9 * i + 9] = bv
            wq_h = w_qkv[8 * h:8 * h + 8] * SCALE
            bq_h = b_qkv[8 * h:8 * h + 8] * SCALE
            wpat_w[:64, i, :] = (rel_w @ wq_h).T
            wpat_w[64, i, :] = rel_w @ bq_h
            wpat_h[:64, i, :] = (rel_h @ wq_h).T
            wpat_h[64, i, :] = rel_h @ bq_h
        wattn_aug = np.zeros((128, 64), np.float32)
        for i, h in enumerate(heads):
            wattn_aug[32 * i + 1:32 * i + 9] = w_attn[:, 8 * h:8 * h + 8].T
        if g == 0:
            wattn_aug[0] += b_attn
        wcat = np.concatenate(
            [wqk_aug, wva_m, wpat_w.reshape(65, 252),
             wpat_h.reshape(65, 252), wconv_aug], 1)  # [65, 668]
        in_maps.append({
            'xbf': np.ascontiguousarray(x_aug.astype(bf)),
            'xc': np.ascontiguousarray(
                x_aug[:, 512 * g:512 * g + 512].astype(bf)),
            'wcat': np.ascontiguousarray(wcat.astype(bf)),
            'wtail': np.ascontiguousarray(wattn_aug.astype(bf)),
            'dconst': DCmat,
        })
    return in_maps


_CACHED = {}


def kernel(**inputs):
    from concourse.bass_utils import run_bass_kernel_spmd
    if 'nc' not in _CACHED:
        _CACHED['nc'] = _build_bass()
    nc = _CACHED['nc']
    in_maps = _host_prep(inputs)
    res = run_bass_kernel_spmd(nc, in_maps, core_ids=list(range(N_CORES)))
    out = np.zeros((B, FILTERS, HW), np.float32)
    for c in range(N_CORES):
        b, g = c // 2, c % 2
        out[b, :64, 512 * g:512 * g + 512] = res.results[c]['o_conv']
        out[b, 64:] += res.results[c]['o_attn']
    return out.reshape(B, FILTERS, H, W)
